# revision 22
# baseline (speedup 1.0000x reference)
"""Bass/Trainium2 kernel for nn_Attention_76338748719569.

Computation (reference):
    attn_input = concat([enc, broadcast(hidden)], dim=2)          [B,T,2H]
    a = tanh(attn_input @ W_hid + b_hid)                          [B,T,H]
    s = a @ w_score                                               [B,T]
    p = softmax(s, axis=T)                                        [B,1,T]
    ctx = p @ enc                                                 [B,1,H]
    return (ctx, p)

with B=32, T=2048, H=1024, fp32.

Strategy: pure data-parallel over batch (4 batches per NeuronCore, 8 cores,
no collectives). Split W_hid into W_e (rows :H, applied to enc) and W_h
(rows H:, applied to hidden): enc@W_e + hidden@W_h == concat@W_hid, which
halves the matmul FLOPs. hidden@W_h + b_hid is a tiny [B,H] tensor folded on
host into the per-(batch, h) bias of the tanh (the "replicate the small
attention MLP weights" part of the sharding hint).

Per core, per batch:
  1. enc tiles [128t, 1024h] are PE-transposed into encT [h,t] bf16 (the
     PSUM->SBUF copy on ScalarE does the cast).
  2. z.T tiles [128h_out, 512t] = sum_k W_e[k,:].T @ encT[k,:] bf16 matmuls,
     fp32 PSUM accumulation.
  3. tanh(z + hb) fused on ScalarE (per-partition bias), output fp32r
     (fp32 rounded to 11 mantissa bits; full PE rate, ~2^-12 rounding).
  4. s[t] += w_score_chunk.T @ aT chunk: M=1 fp32r matvec accumulated in
     PSUM.
  5. softmax over the [1, 2048] score row (max/exp/sum/reciprocal).
  6. expw row is PE-transposed to [128,1] chunks; ctx = sum_j expwT_j.T @
     enc_j streams enc a second time in fp32r (DMA is far from the
     bottleneck); final 1/sum scaling on the way out.
Batches are software-pipelined: batch b+1's transposes are emitted between
batch b's main matmul and its softmax-dependent tail so the PE never waits
on the softmax.
"""

import sys

if "/opt/trn_rl_repo" not in sys.path:
    sys.path.insert(0, "/opt/trn_rl_repo")

import ml_dtypes
import numpy as np

import concourse.bass as bass
import concourse.tile as tile
from concourse import bacc, mybir
from concourse import bass_utils

F32 = mybir.dt.float32
F32R = mybir.dt.float32r
BF16 = mybir.dt.bfloat16
AF = mybir.ActivationFunctionType
AX = mybir.AxisListType

N_CORES = 8
B, T, H = 32, 2048, 1024
NB = B // N_CORES          # batches per core
KC = H // 128              # 8 contraction chunks
HC = H // 128              # 8 h_out chunks
TC = T // 512              # 4 t chunks of 512
TT = T // 128              # 16 t tiles of 128

_COMPILED = None


def _build():
    nc = bacc.Bacc("TRN2", target_bir_lowering=False, debug=False)

    enc_d = nc.dram_tensor("enc", [NB, T, H], BF16, kind="ExternalInput")
    wer_d = nc.dram_tensor("wer", [128, KC * H], BF16, kind="ExternalInput")  # W_e packed [p, k*H+ho]
    hbt_d = nc.dram_tensor("hbt", [128, KC * NB], F32, kind="ExternalInput")  # hb packed [p, k*NB+b]
    wst_d = nc.dram_tensor("wst", [128, KC], F32, kind="ExternalInput")       # w_score packed [p, k]
    idm_d = nc.dram_tensor("idm", [128, 128], F32, kind="ExternalInput")  # identity
    one_d = nc.dram_tensor("one", [1, 128], F32, kind="ExternalInput")    # row of ones
    ctx_d = nc.dram_tensor("ctx", [NB, H], F32, kind="ExternalOutput")
    p_d = nc.dram_tensor("p", [NB, T], F32, kind="ExternalOutput")

    with tile.TileContext(nc) as tc:
        with tc.tile_pool(name="const", bufs=1) as cpool, \
             tc.tile_pool(name="encT", bufs=2) as encT_pool, \
             tc.tile_pool(name="wstage", bufs=2) as stage_pool, \
             tc.tile_pool(name="prod", bufs=2) as prod_pool, \
             tc.tile_pool(name="aT", bufs=10) as aT_pool, \
             tc.tile_pool(name="sm", bufs=1) as sm_pool, \
             tc.tile_pool(name="zps", bufs=4, space="PSUM") as zps_pool, \
             tc.tile_pool(name="smps", bufs=3, space="PSUM") as smps_pool:

            # ---- preamble tiles (loads are emitted inside phase_T0 so the
            # startup transposes and weight loads overlap across the Sync and
            # Scalar HWDGE queues) ------------------------------------------
            ident = cpool.tile([128, 128], F32)
            hbt = cpool.tile([128, KC, NB], F32)
            wst_f = cpool.tile([128, KC], F32)
            wst = cpool.tile([128, KC], F32R)
            we = cpool.tile([128, KC, H], BF16)   # [p, k, h_out]

            warm = cpool.tile([1, 1], F32)

            # per-batch persistent rows (single-buffered; batch phases are
            # ordered M(b) -> S(b) -> C(b) -> M(b+1) so reuse is safe)
            expw = cpool.tile([1, T], F32R)
            sfour = cpool.tile([1, TC], F32)
            rinv = cpool.tile([1, 1], F32)
            pbc = cpool.tile([128, T], BF16)     # expw broadcast across partitions
            ones_f = cpool.tile([1, 128], F32)
            ones = cpool.tile([1, 128], F32R)
            ctxT = [cpool.tile([128, KC], F32, name=f"ctxT{i}", tag=f"ctxT{i}")
                    for i in range(2)]

            encT = [encT_pool.tile([128, KC, T], BF16, name="encT", tag="encT")
                    for i in range(2)]

            # ---- phase helpers -------------------------------------------
            def phase_T0():
                """Startup. Normal HWDGE loads issued strictly before any
                xbar transpose on the same engine (concurrent normal-DMA on a
                second HWDGE engine during transposes corrupts data); the big
                W_e load rides the SWDGE (gpsimd) queue in parallel."""
                nc.gpsimd.dma_start(we[:], wer_d.ap()[:])
                nc.sync.dma_start(hbt[:], hbt_d.ap()[:])
                nc.sync.dma_start(wst_f[:], wst_d.ap()[:])
                nc.sync.dma_start(ident[:], idm_d.ap()[:])
                nc.sync.dma_start(ones_f[:], one_d.ap()[:])
                nc.vector.tensor_copy(wst[:], wst_f[:])
                nc.vector.tensor_copy(ones[:], ones_f[:])
                # warm the ACT table set (exp_and_others holds tanh+exp)
                nc.scalar.activation(warm[:], ident[0:1, 0:1], AF.Tanh)
                # warm the PE HAM clock gate with dummy matmuls while the
                # first encT transposes are still in flight (PE-mode
                # transposes don't count as HAM activity; matmuls do)
                wps = zps_pool.tile([NB, KC * NB], F32, name="wps", tag="wps",
                                    bufs=1)
                for _ in range(64):
                    nc.tensor.matmul(wps[:], hbt[:, 0, :], hbt[:, :, :],
                                     start=True, stop=True,
                                     skip_group_check=True)

                dst = encT[0]

                def q(cq, k):
                    nc.sync.dma_start(
                        dst[:, k, cq * 512:(cq + 1) * 512],
                        enc_d.ap()[0, cq * 512:(cq + 1) * 512, k * 128:(k + 1) * 128],
                        transpose=True)

                for k in range(KC):
                    q(0, k)
                for k in range(KC):
                    q(1, k)
                for k in range(KC):
                    nc.sync.dma_start(
                        dst[:, k, 2 * 512:4 * 512],
                        enc_d.ap()[0, 2 * 512:4 * 512, k * 128:(k + 1) * 128],
                        transpose=True)

            def phase_T(b, quarter=False):
                """Transpose enc[b] into encT[b%2] via the DMA xbar engine.
                T(0) is quartered along t so the first t-chunk's slices land
                early; later batches use one DMA per h_in chunk (per-DMA
                issue on the Sync queue costs 1-3us)."""
                dst = encT[b % 2]
                if quarter:
                    for cq in range(TC):
                        for k in range(KC):
                            nc.sync.dma_start(
                                dst[:, k, cq * 512:(cq + 1) * 512],
                                enc_d.ap()[b, cq * 512:(cq + 1) * 512, k * 128:(k + 1) * 128],
                                transpose=True)
                else:
                    for k in range(KC):
                        nc.sync.dma_start(dst[:, k, :],
                                          enc_d.ap()[b, :, k * 128:(k + 1) * 128],
                                          transpose=True)

            def phase_M(b, finish_prev=None):
                """Main matmuls + tanh + score matvec + per-chunk exp,
                p-broadcast (PE outer product) and ctx partials on VectorE.

                The softmax max-subtraction is dropped: |s| <~ 6 here (tanh
                output dotted with a unit-scale w_score), so exp(s) is safely
                inside fp32 range and softmax is shift-invariant.
                """
                src = encT[b % 2]
                cT = ctxT[b % 2]

                def tail_smv(c, s_ps, aTs):
                    # 8 back-to-back M=1 matvecs pipeline on the PE
                    for h in range(HC):
                        nc.tensor.matmul(s_ps[:], wst[:, h:h + 1], aTs[h][:],
                                         start=(h == 0), stop=(h == HC - 1),
                                         skip_group_check=True)
                    cs = slice(c * 512, (c + 1) * 512)
                    # exp straight from PSUM; per-chunk partial sum
                    nc.scalar.activation(expw[0:1, cs], s_ps[:], AF.Exp)
                    nc.vector.reduce_sum(sfour[0:1, c:c + 1], expw[0:1, cs],
                                         axis=AX.X)

                def tail_ctx(c):
                    cs = slice(c * 512, (c + 1) * 512)
                    # broadcast expw chunk across partitions (PE outer product)
                    pb_ps = smps_pool.tile([128, 512], F32, name="pb_ps", tag="smps")
                    nc.tensor.matmul(pb_ps[:], ones[:], expw[0:1, cs],
                                     start=True, stop=True, skip_group_check=True)
                    nc.scalar.copy(pbc[:, cs], pb_ps[:])
                    # ctx partials on VectorE: ctxT[p,k] += sum_t encT*pbc
                    for k in range(KC):
                        prod = prod_pool.tile([128, 512], BF16)
                        nc.vector.tensor_tensor(prod[:], src[:, k, cs], pbc[:, cs],
                                                op=mybir.AluOpType.mult)
                        if c == 0:
                            nc.vector.reduce_sum(cT[:, k:k + 1], prod[:], axis=AX.X)
                        else:
                            ptmp = sm_pool.tile([128, 1], F32, name="ptmp", tag="ptmp",
                                                bufs=2)
                            nc.vector.reduce_sum(ptmp[:], prod[:], axis=AX.X)
                            nc.vector.tensor_add(cT[:, k:k + 1], cT[:, k:k + 1],
                                                 ptmp[:])

                # pending tails are emitted one h-group later so the PE never
                # waits on ScalarE (tanh/exp) results
                pend = []
                for c in range(TC):
                    s_ps = smps_pool.tile([1, 512], F32, name="s_ps", tag="smps")
                    aTs = [None] * HC
                    for h in range(HC):
                        z_ps = zps_pool.tile([128, 512], F32)
                        for k in range(KC):
                            nc.tensor.matmul(
                                z_ps[:],
                                we[:, k, h * 128:(h + 1) * 128],
                                src[:, k, c * 512:(c + 1) * 512],
                                start=(k == 0), stop=(k == KC - 1),
                            )
                        aT = aT_pool.tile([128, 512], F32R)
                        nc.scalar.activation(aT[:], z_ps[:], AF.Tanh,
                                             bias=hbt[:, h, b:b + 1], scale=1.0)
                        aTs[h] = aT
                        if h == 0 and pend:
                            pend.pop(0)()
                        elif h == 2 and pend:
                            pend.pop(0)()
                        elif h == 4 and c == 0 and finish_prev is not None:
                            finish_prev()
                    pend.append((lambda cc, sp, aa: (lambda: tail_smv(cc, sp, aa)))(c, s_ps, aTs))
                    pend.append((lambda cc: (lambda: tail_ctx(cc)))(c))
                for f in pend:
                    f()

            def phase_S_final(b):
                ssum = sm_pool.tile([1, 1], F32)
                nc.vector.reduce_sum(ssum[:], sfour[:], axis=AX.X)
                nc.vector.reciprocal(rinv[:], ssum[:])
                p_row = sm_pool.tile([1, T], F32)
                nc.vector.tensor_scalar_mul(p_row[:], expw[:], rinv[:, :])
                nc.gpsimd.dma_start(p_d.ap()[b:b + 1, :], p_row[:])

            def ctx_finish(b):
                """Transpose ctxT [128, KC] into a [1, H] row and scale."""
                cT = ctxT[b % 2]
                ctx_ps = [smps_pool.tile([1, 512], F32, name=f"ctx_ps{i2}", tag="smps")
                          for i2 in range(2)]
                for k in range(KC):
                    nc.tensor.transpose(
                        ctx_ps[k // 4][0:1, (k % 4) * 128:(k % 4 + 1) * 128],
                        cT[:, k:k + 1], ident[:])
                ctx_sb = sm_pool.tile([1, H], F32)
                for half in range(2):
                    nc.vector.tensor_scalar_mul(ctx_sb[:, half * 512:(half + 1) * 512],
                                                ctx_ps[half][:], rinv[:, :])
                nc.gpsimd.dma_start(ctx_d.ap()[b:b + 1, :], ctx_sb[:])

            # ---- pipeline ------------------------------------------------
            phase_T0()
            for b in range(NB):
                fp = (lambda bb: (lambda: ctx_finish(bb)))(b - 1) if b > 0 else None
                phase_M(b, finish_prev=fp)
                if b + 1 < NB:
                    phase_T(b + 1)         # xbar transposes run during M(b+1)
                phase_S_final(b)
            ctx_finish(NB - 1)

    nc.compile()
    return nc


def _get_compiled():
    global _COMPILED
    if _COMPILED is None:
        _COMPILED = _build()
    return _COMPILED


def _make_in_maps(hidden, encoder_outputs, W_hid, b_hid, w_score):
    hidden = np.asarray(hidden, dtype=np.float32)
    enc = np.asarray(encoder_outputs, dtype=np.float32)
    W_hid = np.asarray(W_hid, dtype=np.float32)
    b_hid = np.asarray(b_hid, dtype=np.float32)
    w_score = np.asarray(w_score, dtype=np.float32)

    W_e = W_hid[:H]
    W_h = W_hid[H:]
    hb = hidden[:, 0, :] @ W_h + b_hid          # [B, H] tiny, host-side
    werp = np.ascontiguousarray(
        W_e.reshape(KC, 128, H).transpose(1, 0, 2).reshape(128, KC * H)
    ).astype(ml_dtypes.bfloat16)
    wstp = np.ascontiguousarray(w_score.reshape(KC, 128).T)
    idm = np.eye(128, dtype=np.float32)

    in_maps = []
    for c in range(N_CORES):
        sl = slice(c * NB, (c + 1) * NB)
        in_maps.append({
            "enc": np.ascontiguousarray(enc[sl]).astype(ml_dtypes.bfloat16),
            "wer": werp,
            "hbt": np.ascontiguousarray(
                hb[sl].T.reshape(KC, 128, NB).transpose(1, 0, 2).reshape(128, KC * NB)),
            "wst": wstp,
            "idm": idm,
            "one": np.ones((1, 128), dtype=np.float32),
        })
    return in_maps


def _run(in_maps, trace=False, **kw):
    nc = _get_compiled()
    return bass_utils.run_bass_kernel_spmd(
        nc, in_maps, core_ids=list(range(N_CORES)), trace=trace, **kw)


def kernel(hidden, encoder_outputs, W_hid, b_hid, w_score):
    in_maps = _make_in_maps(hidden, encoder_outputs, W_hid, b_hid, w_score)
    res = _run(in_maps, trace=False)
    ctx = np.concatenate([res.results[c]["ctx"] for c in range(N_CORES)], axis=0)
    p = np.concatenate([res.results[c]["p"] for c in range(N_CORES)], axis=0)
    context = ctx.reshape(B, 1, H).astype(np.float32)
    attn_weights = p.reshape(B, 1, T).astype(np.float32)
    return (context, attn_weights)


# revision 24
# speedup vs baseline: 1.0149x; 1.0149x over previous
"""Bass/Trainium2 kernel for nn_Attention_76338748719569.

Computation (reference):
    attn_input = concat([enc, broadcast(hidden)], dim=2)          [B,T,2H]
    a = tanh(attn_input @ W_hid + b_hid)                          [B,T,H]
    s = a @ w_score                                               [B,T]
    p = softmax(s, axis=T)                                        [B,1,T]
    ctx = p @ enc                                                 [B,1,H]
    return (ctx, p)

with B=32, T=2048, H=1024, fp32.

Strategy: pure data-parallel over batch (4 batches per NeuronCore, 8 cores,
no collectives). Split W_hid into W_e (rows :H, applied to enc) and W_h
(rows H:, applied to hidden): enc@W_e + hidden@W_h == concat@W_hid, which
halves the matmul FLOPs. hidden@W_h + b_hid is a tiny [B,H] tensor folded on
host into the per-(batch, h) bias of the tanh (the "replicate the small
attention MLP weights" part of the sharding hint).

Per core, per batch:
  1. enc tiles [128t, 1024h] are PE-transposed into encT [h,t] bf16 (the
     PSUM->SBUF copy on ScalarE does the cast).
  2. z.T tiles [128h_out, 512t] = sum_k W_e[k,:].T @ encT[k,:] bf16 matmuls,
     fp32 PSUM accumulation.
  3. tanh(z + hb) fused on ScalarE (per-partition bias), output fp32r
     (fp32 rounded to 11 mantissa bits; full PE rate, ~2^-12 rounding).
  4. s[t] += w_score_chunk.T @ aT chunk: M=1 fp32r matvec accumulated in
     PSUM.
  5. softmax over the [1, 2048] score row (max/exp/sum/reciprocal).
  6. expw row is PE-transposed to [128,1] chunks; ctx = sum_j expwT_j.T @
     enc_j streams enc a second time in fp32r (DMA is far from the
     bottleneck); final 1/sum scaling on the way out.
Batches are software-pipelined: batch b+1's transposes are emitted between
batch b's main matmul and its softmax-dependent tail so the PE never waits
on the softmax.
"""

import sys

if "/opt/trn_rl_repo" not in sys.path:
    sys.path.insert(0, "/opt/trn_rl_repo")

import ml_dtypes
import numpy as np

import concourse.bass as bass
import concourse.tile as tile
from concourse import bacc, mybir
from concourse import bass_utils

F32 = mybir.dt.float32
F32R = mybir.dt.float32r
BF16 = mybir.dt.bfloat16
AF = mybir.ActivationFunctionType
AX = mybir.AxisListType

N_CORES = 8
B, T, H = 32, 2048, 1024
NB = B // N_CORES          # batches per core
KC = H // 128              # 8 contraction chunks
HC = H // 128              # 8 h_out chunks
TC = T // 512              # 4 t chunks of 512
TT = T // 128              # 16 t tiles of 128

_COMPILED = None


def _build():
    nc = bacc.Bacc("TRN2", target_bir_lowering=False, debug=False)

    enc_d = nc.dram_tensor("enc", [NB, T, H], BF16, kind="ExternalInput")
    wer_d = nc.dram_tensor("wer", [128, KC * H], BF16, kind="ExternalInput")  # W_e packed [p, k*H+ho]
    hbt_d = nc.dram_tensor("hbt", [128, KC * NB], F32, kind="ExternalInput")  # hb packed [p, k*NB+b]
    wst_d = nc.dram_tensor("wst", [128, KC], F32, kind="ExternalInput")       # w_score packed [p, k]
    idm_d = nc.dram_tensor("idm", [128, 128], F32, kind="ExternalInput")  # identity
    one_d = nc.dram_tensor("one", [1, 128], F32, kind="ExternalInput")    # row of ones
    ctx_d = nc.dram_tensor("ctx", [NB, H], F32, kind="ExternalOutput")
    p_d = nc.dram_tensor("p", [NB, T], F32, kind="ExternalOutput")

    with tile.TileContext(nc) as tc:
        with tc.tile_pool(name="const", bufs=1) as cpool, \
             tc.tile_pool(name="encT", bufs=2) as encT_pool, \
             tc.tile_pool(name="wstage", bufs=2) as stage_pool, \
             tc.tile_pool(name="prod", bufs=4) as prod_pool, \
             tc.tile_pool(name="aT", bufs=12) as aT_pool, \
             tc.tile_pool(name="sm", bufs=1) as sm_pool, \
             tc.tile_pool(name="zps", bufs=4, space="PSUM") as zps_pool, \
             tc.tile_pool(name="smps", bufs=3, space="PSUM") as smps_pool:

            # ---- preamble tiles (loads are emitted inside phase_T0 so the
            # startup transposes and weight loads overlap across the Sync and
            # Scalar HWDGE queues) ------------------------------------------
            ident = cpool.tile([128, 128], F32)
            hbt = cpool.tile([128, KC, NB], F32)
            wst_f = cpool.tile([128, KC], F32)
            wst = cpool.tile([128, KC], F32R)
            we = cpool.tile([128, KC, H], BF16)   # [p, k, h_out]

            warm = cpool.tile([1, 1], F32)

            # per-batch persistent rows (single-buffered; batch phases are
            # ordered M(b) -> S(b) -> C(b) -> M(b+1) so reuse is safe)
            expw = cpool.tile([1, T], F32R)
            sfour = cpool.tile([1, TC], F32)
            rinv = cpool.tile([1, 1], F32)
            pbc = cpool.tile([128, T], BF16)     # expw broadcast across partitions
            ones_f = cpool.tile([1, 128], F32)
            ones = cpool.tile([1, 128], F32R)
            ctxT = [cpool.tile([128, KC], F32, name=f"ctxT{i}", tag=f"ctxT{i}")
                    for i in range(2)]

            encT = [encT_pool.tile([128, KC, T], BF16, name="encT", tag="encT")
                    for i in range(2)]

            # ---- phase helpers -------------------------------------------
            def phase_T0():
                """Startup. Normal HWDGE loads issued strictly before any
                xbar transpose on the same engine (concurrent normal-DMA on a
                second HWDGE engine during transposes corrupts data); the big
                W_e load rides the SWDGE (gpsimd) queue in parallel."""
                nc.gpsimd.dma_start(we[:], wer_d.ap()[:])
                nc.sync.dma_start(hbt[:], hbt_d.ap()[:])
                nc.sync.dma_start(wst_f[:], wst_d.ap()[:])
                nc.sync.dma_start(ident[:], idm_d.ap()[:])
                nc.sync.dma_start(ones_f[:], one_d.ap()[:])
                nc.vector.tensor_copy(wst[:], wst_f[:])
                nc.vector.tensor_copy(ones[:], ones_f[:])
                # warm the ACT table set (exp_and_others holds tanh+exp)
                nc.scalar.activation(warm[:], ident[0:1, 0:1], AF.Tanh)
                # warm the PE HAM clock gate with dummy matmuls while the
                # first encT transposes are still in flight (PE-mode
                # transposes don't count as HAM activity; matmuls do)
                wps = zps_pool.tile([NB, KC * NB], F32, name="wps", tag="wps",
                                    bufs=1)
                for _ in range(32):
                    nc.tensor.matmul(wps[:], hbt[:, 0, :], hbt[:, :, :],
                                     start=True, stop=True,
                                     skip_group_check=True)

                dst = encT[0]

                def q(cq, k):
                    nc.sync.dma_start(
                        dst[:, k, cq * 512:(cq + 1) * 512],
                        enc_d.ap()[0, cq * 512:(cq + 1) * 512, k * 128:(k + 1) * 128],
                        transpose=True)

                for k in range(KC):
                    q(0, k)
                for k in range(KC):
                    q(1, k)
                for k in range(KC):
                    nc.sync.dma_start(
                        dst[:, k, 2 * 512:4 * 512],
                        enc_d.ap()[0, 2 * 512:4 * 512, k * 128:(k + 1) * 128],
                        transpose=True)

            def phase_T(b, quarter=False):
                """Transpose enc[b] into encT[b%2] via the DMA xbar engine.
                T(0) is quartered along t so the first t-chunk's slices land
                early; later batches use one DMA per h_in chunk (per-DMA
                issue on the Sync queue costs 1-3us)."""
                dst = encT[b % 2]
                if quarter:
                    for cq in range(TC):
                        for k in range(KC):
                            nc.sync.dma_start(
                                dst[:, k, cq * 512:(cq + 1) * 512],
                                enc_d.ap()[b, cq * 512:(cq + 1) * 512, k * 128:(k + 1) * 128],
                                transpose=True)
                else:
                    for k in range(KC):
                        nc.sync.dma_start(dst[:, k, :],
                                          enc_d.ap()[b, :, k * 128:(k + 1) * 128],
                                          transpose=True)

            def phase_M(b, finish_prev=None):
                """Main matmuls + tanh + score matvec + per-chunk exp,
                p-broadcast (PE outer product) and ctx partials on VectorE.

                The softmax max-subtraction is dropped: |s| <~ 6 here (tanh
                output dotted with a unit-scale w_score), so exp(s) is safely
                inside fp32 range and softmax is shift-invariant.
                """
                src = encT[b % 2]
                cT = ctxT[b % 2]

                def tail_smv(c, s_ps, aTs):
                    # 8 back-to-back M=1 matvecs pipeline on the PE
                    for h in range(HC):
                        nc.tensor.matmul(s_ps[:], wst[:, h:h + 1], aTs[h][:],
                                         start=(h == 0), stop=(h == HC - 1),
                                         skip_group_check=True)
                    cs = slice(c * 512, (c + 1) * 512)
                    # exp straight from PSUM; per-chunk partial sum
                    nc.scalar.activation(expw[0:1, cs], s_ps[:], AF.Exp)
                    nc.vector.reduce_sum(sfour[0:1, c:c + 1], expw[0:1, cs],
                                         axis=AX.X)

                def tail_ctx(c):
                    cs = slice(c * 512, (c + 1) * 512)
                    # broadcast expw chunk across partitions (PE outer product)
                    pb_ps = smps_pool.tile([128, 512], F32, name="pb_ps", tag="smps")
                    nc.tensor.matmul(pb_ps[:], ones[:], expw[0:1, cs],
                                     start=True, stop=True, skip_group_check=True)
                    nc.scalar.copy(pbc[:, cs], pb_ps[:])
                    # ctx partials on VectorE: ctxT[p,k] += sum_t encT*pbc
                    for k in range(KC):
                        prod = prod_pool.tile([128, 512], BF16)
                        nc.vector.tensor_tensor(prod[:], src[:, k, cs], pbc[:, cs],
                                                op=mybir.AluOpType.mult)
                        if c == 0:
                            nc.vector.reduce_sum(cT[:, k:k + 1], prod[:], axis=AX.X)
                        else:
                            ptmp = sm_pool.tile([128, 1], F32, name="ptmp", tag="ptmp",
                                                bufs=2)
                            nc.vector.reduce_sum(ptmp[:], prod[:], axis=AX.X)
                            nc.vector.tensor_add(cT[:, k:k + 1], cT[:, k:k + 1],
                                                 ptmp[:])

                # pending tails are emitted one h-group later so the PE never
                # waits on ScalarE (tanh/exp) results
                pend = []
                for c in range(TC):
                    s_ps = smps_pool.tile([1, 512], F32, name="s_ps", tag="smps")
                    aTs = [None] * HC
                    for h in range(HC):
                        z_ps = zps_pool.tile([128, 512], F32)
                        for k in range(KC):
                            nc.tensor.matmul(
                                z_ps[:],
                                we[:, k, h * 128:(h + 1) * 128],
                                src[:, k, c * 512:(c + 1) * 512],
                                start=(k == 0), stop=(k == KC - 1),
                            )
                        aT = aT_pool.tile([128, 512], F32R)
                        nc.scalar.activation(aT[:], z_ps[:], AF.Tanh,
                                             bias=hbt[:, h, b:b + 1], scale=1.0)
                        aTs[h] = aT
                        if h == 1 and pend:
                            pend.pop(0)()
                        elif h == 3 and pend:
                            pend.pop(0)()
                        elif h == 5 and c == 0 and finish_prev is not None:
                            finish_prev()
                    pend.append((lambda cc, sp, aa: (lambda: tail_smv(cc, sp, aa)))(c, s_ps, aTs))
                    pend.append((lambda cc: (lambda: tail_ctx(cc)))(c))
                for f in pend:
                    f()

            def phase_S_final(b):
                ssum = sm_pool.tile([1, 1], F32)
                nc.vector.reduce_sum(ssum[:], sfour[:], axis=AX.X)
                nc.vector.reciprocal(rinv[:], ssum[:])
                p_row = sm_pool.tile([1, T], F32)
                nc.vector.tensor_scalar_mul(p_row[:], expw[:], rinv[:, :])
                nc.gpsimd.dma_start(p_d.ap()[b:b + 1, :], p_row[:])

            def ctx_finish(b):
                """Transpose ctxT [128, KC] into a [1, H] row and scale."""
                cT = ctxT[b % 2]
                ctx_ps = [smps_pool.tile([1, 512], F32, name=f"ctx_ps{i2}", tag="smps")
                          for i2 in range(2)]
                for k in range(KC):
                    nc.tensor.transpose(
                        ctx_ps[k // 4][0:1, (k % 4) * 128:(k % 4 + 1) * 128],
                        cT[:, k:k + 1], ident[:])
                ctx_sb = sm_pool.tile([1, H], F32)
                for half in range(2):
                    nc.vector.tensor_scalar_mul(ctx_sb[:, half * 512:(half + 1) * 512],
                                                ctx_ps[half][:], rinv[:, :])
                nc.gpsimd.dma_start(ctx_d.ap()[b:b + 1, :], ctx_sb[:])

            # ---- pipeline ------------------------------------------------
            phase_T0()
            for b in range(NB):
                fp = (lambda bb: (lambda: ctx_finish(bb)))(b - 1) if b > 0 else None
                phase_M(b, finish_prev=fp)
                if b + 1 < NB:
                    phase_T(b + 1)         # xbar transposes run during M(b+1)
                phase_S_final(b)
            ctx_finish(NB - 1)

    nc.compile()
    return nc


def _get_compiled():
    global _COMPILED
    if _COMPILED is None:
        _COMPILED = _build()
    return _COMPILED


def _make_in_maps(hidden, encoder_outputs, W_hid, b_hid, w_score):
    hidden = np.asarray(hidden, dtype=np.float32)
    enc = np.asarray(encoder_outputs, dtype=np.float32)
    W_hid = np.asarray(W_hid, dtype=np.float32)
    b_hid = np.asarray(b_hid, dtype=np.float32)
    w_score = np.asarray(w_score, dtype=np.float32)

    W_e = W_hid[:H]
    W_h = W_hid[H:]
    hb = hidden[:, 0, :] @ W_h + b_hid          # [B, H] tiny, host-side
    werp = np.ascontiguousarray(
        W_e.reshape(KC, 128, H).transpose(1, 0, 2).reshape(128, KC * H)
    ).astype(ml_dtypes.bfloat16)
    wstp = np.ascontiguousarray(w_score.reshape(KC, 128).T)
    idm = np.eye(128, dtype=np.float32)

    in_maps = []
    for c in range(N_CORES):
        sl = slice(c * NB, (c + 1) * NB)
        in_maps.append({
            "enc": np.ascontiguousarray(enc[sl]).astype(ml_dtypes.bfloat16),
            "wer": werp,
            "hbt": np.ascontiguousarray(
                hb[sl].T.reshape(KC, 128, NB).transpose(1, 0, 2).reshape(128, KC * NB)),
            "wst": wstp,
            "idm": idm,
            "one": np.ones((1, 128), dtype=np.float32),
        })
    return in_maps


def _run(in_maps, trace=False, **kw):
    nc = _get_compiled()
    return bass_utils.run_bass_kernel_spmd(
        nc, in_maps, core_ids=list(range(N_CORES)), trace=trace, **kw)


def kernel(hidden, encoder_outputs, W_hid, b_hid, w_score):
    in_maps = _make_in_maps(hidden, encoder_outputs, W_hid, b_hid, w_score)
    res = _run(in_maps, trace=False)
    ctx = np.concatenate([res.results[c]["ctx"] for c in range(N_CORES)], axis=0)
    p = np.concatenate([res.results[c]["p"] for c in range(N_CORES)], axis=0)
    context = ctx.reshape(B, 1, H).astype(np.float32)
    attn_weights = p.reshape(B, 1, T).astype(np.float32)
    return (context, attn_weights)
